# revision 23
# baseline (speedup 1.0000x reference)
"""GCN-with-edge-features kernel for 8 Trainium2 cores.

The per-edge weight matrices theta = relu(ea@Wa+ba)@Wb+bb depend ONLY
on the 16-bit edge attribute, and E=100k random edges hit only ~51.3k
distinct values. The device computes theta once per UNIQUE attribute
(0.51x the dominant GEMM FLOPs) as pure fp8 DoubleRow matmul work:

  per 128-unique tile: 4*nb DR matmuls (K=1024, N=512) -> PSUM,
  ScalarE compresses fp32 -> bf16 SBUF, DMA streams theta to HBM.

All three stages pipeline under the tensor engine (scalar 2.0us, DMA
1.5us vs matmul 3.6us per L2 tile), so each launch runs at the fp8
matmul roofline. Stage A of the edge-net (K=16 GEMM over unique attrs,
~2 GFLOP) runs on host, uploading h = relu(ea_u@Wa+ba) as fp8 in
matmul lhsT layout; replicated Wb uploads once per layer.

The per-edge contraction msg[e] = x[src[e]] @ theta[u(e)] is only
0.4 GFLOP total (800x less than the GEMMs) and runs on host as
count-class-batched matmuls over zero-copy theta views (unique slots
are emitted in count-sorted order), like the segment-mean aggregation,
graph pooling and FC head already do.

theta2 depends only on the edge attributes (not on layer-1 output), so
BOTH layers' theta GEMMs run in a single launch.
"""
import numpy as np

import sys
for p in ("/opt/trn_rl_repo",):
    if p not in sys.path:
        sys.path.append(p)

import ml_dtypes

from concourse import bass, bacc, mybir, tile
from concourse import bass_utils

E = 100000
N = 50000
NG = 2000
F_IN = 32
EF = 16
H = 32
H2 = 64
NC = 8

_F32 = mybir.dt.float32
_BF16 = mybir.dt.bfloat16
_F8 = mybir.dt.float8e4
_COPY = mybir.ActivationFunctionType.Copy
_DR = mybir.MatmulPerfMode.DoubleRow

_NC_CACHE = {}
_WB_CACHE = {}
LAST_RUNS = []  # BassKernelResults of the device launches in the last kernel() call

BF16 = ml_dtypes.bfloat16
F8E4 = ml_dtypes.float8_e4m3fn


def _build_pass(ntile):
    """Both GNN layers' theta GEMMs in one launch: theta1 [S, 1024] then
    theta2 [S, 2048] for S = ntile*128 unique slots per core. theta2
    depends only on the edge attributes, not on layer-1 output, so both
    layers share one kernel (one fill + drain instead of two)."""
    S = ntile * 128

    nc = bacc.Bacc(None, target_bir_lowering=False)
    hT1_d = nc.dram_tensor("hT1", [128, 8 * S], _F8, kind="ExternalInput")
    hT2_d = nc.dram_tensor("hT2", [128, 8 * S], _F8, kind="ExternalInput")
    Wb1_d = nc.dram_tensor("Wb1", [128, 8 * 1024], _F8, kind="ExternalInput")
    Wb2_d = nc.dram_tensor("Wb2", [128, 8 * 2048], _F8, kind="ExternalInput")
    tho1_d = nc.dram_tensor("tho1", [128, ntile * 1024], _BF16,
                            kind="ExternalOutput")
    tho2_d = nc.dram_tensor("tho2", [128, ntile * 2048], _BF16,
                            kind="ExternalOutput")

    with tile.TileContext(nc) as tc:
        with (
            tc.tile_pool(name="w", bufs=1) as wpool,
            tc.tile_pool(name="ths", bufs=8) as thspool,
            tc.tile_pool(name="th", bufs=2,
                         space=bass.MemorySpace.PSUM) as thpool,
        ):
            # layer-1 weights + first h1 chunk land first; the bulk h
            # streams ride the Activation hardware-DGE queue (triggers
            # enqueue before the tile-loop ACTIVATEs) so per-tile theta
            # stores on the SP queue never wait behind them.
            wb1 = wpool.tile([128, 4, 2, 2, 512], _F8)
            for bk in range(2):
                nc.sync.dma_start(wb1[:, :, bk], Wb1_d[:].rearrange(
                    "p (a b c e) -> p a b (c e)", a=4, b=2, c=2)[:, :, bk])
            hT1 = wpool.tile([128, 8, S], _F8)
            nc.sync.dma_start(hT1[:, :, :128], hT1_d[:].rearrange(
                "p (j s) -> p j s", j=8)[:, :, :128])
            hT2 = wpool.tile([128, 8, S], _F8)
            wb2 = wpool.tile([128, 4, 4, 2, 512], _F8)
            # bulk streams, issued one-per-tile inside the loop so the
            # Activation HWDGE ring never backpressures the ACTIVATEs
            bulk = []
            for a in range(128, S, 512):
                b = min(a + 512, S)
                bulk.append((hT1[:, :, a:b], hT1_d[:].rearrange(
                    "p (j s) -> p j s", j=8)[:, :, a:b]))
            for bk in range(4):
                bulk.append((wb2[:, :, bk], Wb2_d[:].rearrange(
                    "p (a b c e) -> p a b (c e)", a=4, b=4, c=2)[:, :, bk]))
            for a in range(0, S, 512):
                b = min(a + 512, S)
                bulk.append((hT2[:, :, a:b], hT2_d[:].rearrange(
                    "p (j s) -> p j s", j=8)[:, :, a:b]))
            bulk.reverse()

            for fo, hT, wb, tho_d in ((32, hT1, wb1, tho1_d),
                                      (64, hT2, wb2, tho2_d)):
                d = 32 * fo
                nb = d // 512
                for tg in range(ntile):
                    th = thpool.tile([128, 2048], _F32, name="th", tag="th")
                    for b in range(nb):
                        for jp in range(4):
                            nc.tensor.matmul(
                                th[:, b * 512:(b + 1) * 512],
                                hT[:, 2 * jp:2 * jp + 2,
                                   tg * 128:(tg + 1) * 128],
                                wb[:, jp, b, :, :],
                                start=(jp == 0), stop=(jp == 3),
                                perf_mode=_DR,
                            )
                    if bulk:
                        dst, srcap = bulk.pop()
                        nc.scalar.dma_start(dst, srcap)
                    ths = thspool.tile([128, 2048], _BF16)
                    nc.scalar.activation(ths[:, :d], th[:, :d], _COPY)
                    nc.sync.dma_start(tho_d[:, tg * d:(tg + 1) * d],
                                      ths[:, :d])

    nc.compile()
    return nc


def _get_nc(ntile):
    if ntile not in _NC_CACHE:
        _NC_CACHE[ntile] = _build_pass(ntile)
    return _NC_CACHE[ntile]


def _relu(v):
    return np.maximum(v, 0.0)


class _SegMean:
    """Sort-based segment mean (np.add.at is too slow)."""

    def __init__(self, idx, n):
        self.n = n
        self.order = np.argsort(idx, kind="stable")
        sorted_idx = np.asarray(idx)[self.order]
        self.uniq, self.starts = np.unique(sorted_idx, return_index=True)
        self.cnt = np.maximum(
            np.bincount(np.asarray(idx), minlength=n), 1.0
        ).astype(np.float32)[:, None]

    def __call__(self, vals):
        out = np.zeros((self.n, vals.shape[1]), np.float32)
        out[self.uniq] = np.add.reduceat(vals[self.order], self.starts, axis=0)
        return out / self.cnt


class _Schedule:
    """Dedup bookkeeping shared by both layers.

    Unique slots are count-sorted so each count class is a contiguous
    slot range; the device emits theta in slot order, letting the host
    contraction take zero-copy per-class theta views.
    """

    def __init__(self, eap):
        v = (eap[:, 0].astype(np.int64) << 8) | eap[:, 1].astype(np.int64)
        uniq, inv, counts = np.unique(v, return_inverse=True,
                                      return_counts=True)
        self.uniq_vals = uniq
        U = len(uniq)
        edge_order = np.argsort(inv, kind="stable").astype(np.int64)
        starts = np.zeros(U + 1, np.int64)
        np.cumsum(counts, out=starts[1:])

        order_u = np.argsort(counts, kind="stable")   # uniques by count
        sc = counts[order_u]
        self.ntile = -(-U // (NC * 128))
        S = self.ntile * NC * 128
        slot_uid = np.concatenate([order_u, np.full(S - U, -1, np.int64)])
        # slot s -> (tile s//1024, core (s//128)%8, partition s%128)
        self.core_uid = slot_uid.reshape(self.ntile, NC, 128).transpose(1, 0, 2) \
            .reshape(NC, -1)

        # per count class: slot range [lo, hi) and edge-id matrix [n, c]
        self.classes = []
        lo = 0
        for c in np.unique(sc):
            hi = int(np.searchsorted(sc, c, side="right"))
            us = order_u[lo:hi]
            em = edge_order[starts[us][:, None] + np.arange(c)]
            self.classes.append((int(c), lo, hi, em))
            lo = hi
        self.S = S

    def contract(self, theta_slots, xfull, Bb, fo):
        """msg[e] = xfull[e] @ theta[u(e)] + xfull[e] @ Bb."""
        msg = xfull @ Bb.reshape(32, fo)
        for c, lo, hi, em in self.classes:
            out = np.matmul(xfull[em], theta_slots[lo:hi])  # [n, c, fo]
            msg[em.reshape(-1)] += out.reshape(-1, fo)
        return msg


def _pack_hT(sch, h_u):
    """[U, 1024] fp32 -> per-core fp8 lhsT layout [128, 8*S]."""
    h_u8 = h_u.astype(F8E4)
    U = h_u8.shape[0]
    outs = []
    for k in range(NC):
        uid = sch.core_uid[k]
        hs = h_u8[np.minimum(uid, U - 1)]
        hs[uid < 0] = 0
        S = hs.shape[0]
        outs.append(np.ascontiguousarray(
            hs.T.reshape(8, 128, S).transpose(1, 0, 2).reshape(128, 8 * S)))
    return outs


def _theta_slots(sch, res, name, fo):
    """[core][128, ntile*d] bf16 -> [S, 32, fo] fp32 in slot order."""
    theta = np.empty((sch.ntile, NC, 128, 32, fo), np.float32)
    for k in range(NC):
        m = np.asarray(res.results[k][name]).astype(np.float32)
        theta[:, k] = m.reshape(128, sch.ntile, 32, fo).transpose(1, 0, 2, 3)
    return theta.reshape(sch.S, 32, fo)


def _run_both(sch, h1_u, h2_u):
    """One launch computing theta1 and theta2 for all unique slots."""
    nc = _get_nc(sch.ntile)
    hT1 = _pack_hT(sch, h1_u)
    hT2 = _pack_hT(sch, h2_u)
    in_maps = [dict(hT1=hT1[k], hT2=hT2[k], Wb1=_WB_CACHE[32],
                    Wb2=_WB_CACHE[64]) for k in range(NC)]
    res = bass_utils.run_bass_kernel_spmd(nc, in_maps, core_ids=list(range(NC)))
    LAST_RUNS.append(res)
    return res


def _pack_wb(fo, Wb):
    # [k=1024, d] -> [p, jp, bank, plane, n]; k = (2*jp+plane)*128+p
    d = 32 * fo
    nb = d // 512
    _WB_CACHE[fo] = np.ascontiguousarray(
        Wb.reshape(4, 2, 128, nb, 512).transpose(2, 0, 3, 1, 4)
        .reshape(128, 8 * d)).astype(F8E4)


def kernel(**inputs):
    x = np.asarray(inputs["x"], np.float32)
    edge_index = np.asarray(inputs["edge_index"])
    eap = np.asarray(inputs["edge_attr_packed"])
    batch = np.asarray(inputs["batch"])
    W1a = np.asarray(inputs["W1a"], np.float32)
    W1b = np.asarray(inputs["W1b"], np.float32)
    W2a = np.asarray(inputs["W2a"], np.float32)
    W2b = np.asarray(inputs["W2b"], np.float32)
    b1a = np.asarray(inputs["b1a"], np.float32)
    b1b = np.asarray(inputs["b1b"], np.float32)
    b2a = np.asarray(inputs["b2a"], np.float32)
    b2b = np.asarray(inputs["b2b"], np.float32)
    root1 = np.asarray(inputs["root1"], np.float32)
    bias1 = np.asarray(inputs["bias1"], np.float32)
    root2 = np.asarray(inputs["root2"], np.float32)
    bias2 = np.asarray(inputs["bias2"], np.float32)

    LAST_RUNS.clear()
    sch = _Schedule(eap)
    _pack_wb(32, W1b)
    _pack_wb(64, W2b)

    # unique edge-attr bit patterns -> [U, 16] (MSB-first per byte)
    shifts = np.arange(15, -1, -1, dtype=np.int64)
    ea_u = ((sch.uniq_vals[:, None] >> shifts) & 1).astype(np.float32)

    src, dst = edge_index[0], edge_index[1]
    segmean_dst = _SegMean(dst, N)

    h1_u = _relu(ea_u @ W1a + b1a)
    h2_u = _relu(ea_u @ W2a + b2a)
    res = _run_both(sch, h1_u, h2_u)

    theta1 = _theta_slots(sch, res, "tho1", 32)
    msg1 = sch.contract(theta1, x[src], b1b, 32)
    h = _relu(segmean_dst(msg1) + x @ root1 + bias1)

    theta2 = _theta_slots(sch, res, "tho2", 64)
    msg2 = sch.contract(theta2, h[src], b2b, 64)
    h = _relu(segmean_dst(msg2) + h @ root2 + bias2)

    g = _SegMean(batch, NG)(h)
    g = _relu(g @ np.asarray(inputs["fcW1"], np.float32) + np.asarray(inputs["fcb1"], np.float32))
    g = _relu(g @ np.asarray(inputs["fcW2"], np.float32) + np.asarray(inputs["fcb2"], np.float32))
    g = _relu(g @ np.asarray(inputs["fcW3"], np.float32) + np.asarray(inputs["fcb3"], np.float32))
    return (g @ np.asarray(inputs["fcW4"], np.float32) + np.asarray(inputs["fcb4"], np.float32)).astype(np.float32)


# revision 24
# speedup vs baseline: 1.0059x; 1.0059x over previous
"""GCN-with-edge-features kernel for 8 Trainium2 cores.

The per-edge weight matrices theta = relu(ea@Wa+ba)@Wb+bb depend ONLY
on the 16-bit edge attribute, and E=100k random edges hit only ~51.3k
distinct values. The device computes theta once per UNIQUE attribute
(0.51x the dominant GEMM FLOPs) as pure fp8 DoubleRow matmul work:

  per 128-unique tile: 4*nb DR matmuls (K=1024, N=512) -> PSUM,
  ScalarE compresses fp32 -> bf16 SBUF, DMA streams theta to HBM.

All three stages pipeline under the tensor engine (scalar 2.0us, DMA
1.5us vs matmul 3.6us per L2 tile), so each launch runs at the fp8
matmul roofline. Stage A of the edge-net (K=16 GEMM over unique attrs,
~2 GFLOP) runs on host, uploading h = relu(ea_u@Wa+ba) as fp8 in
matmul lhsT layout; replicated Wb uploads once per layer.

The per-edge contraction msg[e] = x[src[e]] @ theta[u(e)] is only
0.4 GFLOP total (800x less than the GEMMs) and runs on host as
count-class-batched matmuls over zero-copy theta views (unique slots
are emitted in count-sorted order), like the segment-mean aggregation,
graph pooling and FC head already do.

theta2 depends only on the edge attributes (not on layer-1 output), so
BOTH layers' theta GEMMs run in a single launch.
"""
import numpy as np

import sys
for p in ("/opt/trn_rl_repo",):
    if p not in sys.path:
        sys.path.append(p)

import ml_dtypes

from concourse import bass, bacc, mybir, tile
from concourse import bass_utils

E = 100000
N = 50000
NG = 2000
F_IN = 32
EF = 16
H = 32
H2 = 64
NC = 8

_F32 = mybir.dt.float32
_BF16 = mybir.dt.bfloat16
_F8 = mybir.dt.float8e4
_COPY = mybir.ActivationFunctionType.Copy
_DR = mybir.MatmulPerfMode.DoubleRow

_NC_CACHE = {}
_WB_CACHE = {}
LAST_RUNS = []  # BassKernelResults of the device launches in the last kernel() call

BF16 = ml_dtypes.bfloat16
F8E4 = ml_dtypes.float8_e4m3fn


def _build_pass(ntile):
    """Both GNN layers' theta GEMMs in one launch: theta1 [S, 1024] then
    theta2 [S, 2048] for S = ntile*128 unique slots per core. theta2
    depends only on the edge attributes, not on layer-1 output, so both
    layers share one kernel (one fill + drain instead of two)."""
    S = ntile * 128

    nc = bacc.Bacc(None, target_bir_lowering=False)
    hT1_d = nc.dram_tensor("hT1", [128, 8 * S], _F8, kind="ExternalInput")
    hT2_d = nc.dram_tensor("hT2", [128, 8 * S], _F8, kind="ExternalInput")
    Wb1_d = nc.dram_tensor("Wb1", [128, 8 * 1024], _F8, kind="ExternalInput")
    Wb2_d = nc.dram_tensor("Wb2", [128, 8 * 2048], _F8, kind="ExternalInput")
    tho1_d = nc.dram_tensor("tho1", [128, ntile * 1024], _BF16,
                            kind="ExternalOutput")
    tho2_d = nc.dram_tensor("tho2", [128, ntile * 2048], _BF16,
                            kind="ExternalOutput")

    with tile.TileContext(nc) as tc:
        with (
            tc.tile_pool(name="w", bufs=1) as wpool,
            tc.tile_pool(name="ths", bufs=6) as thspool,
            tc.tile_pool(name="th", bufs=2,
                         space=bass.MemorySpace.PSUM) as thpool,
        ):
            # layer-1 weights + first h1 chunk land first; the bulk h
            # streams ride the Activation hardware-DGE queue (triggers
            # enqueue before the tile-loop ACTIVATEs) so per-tile theta
            # stores on the SP queue never wait behind them.
            wb1 = wpool.tile([128, 4, 2, 2, 512], _F8)
            nc.sync.dma_start(wb1[:].rearrange("p a b c e -> p (a b c e)"),
                              Wb1_d[:])
            hT1 = wpool.tile([128, 8, S], _F8)
            nc.sync.dma_start(hT1[:, :, :128], hT1_d[:].rearrange(
                "p (j s) -> p j s", j=8)[:, :, :128])
            hT2 = wpool.tile([128, 8, S], _F8)
            wb2 = wpool.tile([128, 4, 4, 2, 512], _F8)
            # bulk streams, issued one-per-tile inside the loop so the
            # Activation HWDGE ring never backpressures the ACTIVATEs
            bulk = []
            for a in range(128, S, 512):
                b = min(a + 512, S)
                bulk.append((hT1[:, :, a:b], hT1_d[:].rearrange(
                    "p (j s) -> p j s", j=8)[:, :, a:b]))
            for bk in range(4):
                bulk.append((wb2[:, :, bk], Wb2_d[:].rearrange(
                    "p (a b c e) -> p a b (c e)", a=4, b=4, c=2)[:, :, bk]))
            for a in range(0, S, 512):
                b = min(a + 512, S)
                bulk.append((hT2[:, :, a:b], hT2_d[:].rearrange(
                    "p (j s) -> p j s", j=8)[:, :, a:b]))
            bulk.reverse()

            for fo, hT, wb, tho_d in ((32, hT1, wb1, tho1_d),
                                      (64, hT2, wb2, tho2_d)):
                d = 32 * fo
                nb = d // 512
                for tg in range(ntile):
                    th = thpool.tile([128, 2048], _F32, name="th", tag="th")
                    for b in range(nb):
                        for jp in range(4):
                            nc.tensor.matmul(
                                th[:, b * 512:(b + 1) * 512],
                                hT[:, 2 * jp:2 * jp + 2,
                                   tg * 128:(tg + 1) * 128],
                                wb[:, jp, b, :, :],
                                start=(jp == 0), stop=(jp == 3),
                                perf_mode=_DR,
                            )
                    if bulk:
                        dst, srcap = bulk.pop()
                        nc.scalar.dma_start(dst, srcap)
                    ths = thspool.tile([128, 2048], _BF16)
                    nc.scalar.activation(ths[:, :d], th[:, :d], _COPY)
                    nc.sync.dma_start(tho_d[:, tg * d:(tg + 1) * d],
                                      ths[:, :d])

    nc.compile()
    return nc


def _get_nc(ntile):
    if ntile not in _NC_CACHE:
        _NC_CACHE[ntile] = _build_pass(ntile)
    return _NC_CACHE[ntile]


def _relu(v):
    return np.maximum(v, 0.0)


class _SegMean:
    """Sort-based segment mean (np.add.at is too slow)."""

    def __init__(self, idx, n):
        self.n = n
        self.order = np.argsort(idx, kind="stable")
        sorted_idx = np.asarray(idx)[self.order]
        self.uniq, self.starts = np.unique(sorted_idx, return_index=True)
        self.cnt = np.maximum(
            np.bincount(np.asarray(idx), minlength=n), 1.0
        ).astype(np.float32)[:, None]

    def __call__(self, vals):
        out = np.zeros((self.n, vals.shape[1]), np.float32)
        out[self.uniq] = np.add.reduceat(vals[self.order], self.starts, axis=0)
        return out / self.cnt


class _Schedule:
    """Dedup bookkeeping shared by both layers.

    Unique slots are count-sorted so each count class is a contiguous
    slot range; the device emits theta in slot order, letting the host
    contraction take zero-copy per-class theta views.
    """

    def __init__(self, eap):
        v = (eap[:, 0].astype(np.int64) << 8) | eap[:, 1].astype(np.int64)
        uniq, inv, counts = np.unique(v, return_inverse=True,
                                      return_counts=True)
        self.uniq_vals = uniq
        U = len(uniq)
        edge_order = np.argsort(inv, kind="stable").astype(np.int64)
        starts = np.zeros(U + 1, np.int64)
        np.cumsum(counts, out=starts[1:])

        order_u = np.argsort(counts, kind="stable")   # uniques by count
        sc = counts[order_u]
        self.ntile = -(-U // (NC * 128))
        S = self.ntile * NC * 128
        slot_uid = np.concatenate([order_u, np.full(S - U, -1, np.int64)])
        # slot s -> (tile s//1024, core (s//128)%8, partition s%128)
        self.core_uid = slot_uid.reshape(self.ntile, NC, 128).transpose(1, 0, 2) \
            .reshape(NC, -1)

        # per count class: slot range [lo, hi) and edge-id matrix [n, c]
        self.classes = []
        lo = 0
        for c in np.unique(sc):
            hi = int(np.searchsorted(sc, c, side="right"))
            us = order_u[lo:hi]
            em = edge_order[starts[us][:, None] + np.arange(c)]
            self.classes.append((int(c), lo, hi, em))
            lo = hi
        self.S = S

    def contract(self, theta_slots, xfull, Bb, fo):
        """msg[e] = xfull[e] @ theta[u(e)] + xfull[e] @ Bb."""
        msg = xfull @ Bb.reshape(32, fo)
        for c, lo, hi, em in self.classes:
            out = np.matmul(xfull[em], theta_slots[lo:hi])  # [n, c, fo]
            msg[em.reshape(-1)] += out.reshape(-1, fo)
        return msg


def _pack_hT(sch, h_u):
    """[U, 1024] fp32 -> per-core fp8 lhsT layout [128, 8*S]."""
    h_u8 = h_u.astype(F8E4)
    U = h_u8.shape[0]
    outs = []
    for k in range(NC):
        uid = sch.core_uid[k]
        hs = h_u8[np.minimum(uid, U - 1)]
        hs[uid < 0] = 0
        S = hs.shape[0]
        outs.append(np.ascontiguousarray(
            hs.T.reshape(8, 128, S).transpose(1, 0, 2).reshape(128, 8 * S)))
    return outs


def _theta_slots(sch, res, name, fo):
    """[core][128, ntile*d] bf16 -> [S, 32, fo] fp32 in slot order."""
    theta = np.empty((sch.ntile, NC, 128, 32, fo), np.float32)
    for k in range(NC):
        m = np.asarray(res.results[k][name]).astype(np.float32)
        theta[:, k] = m.reshape(128, sch.ntile, 32, fo).transpose(1, 0, 2, 3)
    return theta.reshape(sch.S, 32, fo)


def _run_both(sch, h1_u, h2_u):
    """One launch computing theta1 and theta2 for all unique slots."""
    nc = _get_nc(sch.ntile)
    hT1 = _pack_hT(sch, h1_u)
    hT2 = _pack_hT(sch, h2_u)
    in_maps = [dict(hT1=hT1[k], hT2=hT2[k], Wb1=_WB_CACHE[32],
                    Wb2=_WB_CACHE[64]) for k in range(NC)]
    res = bass_utils.run_bass_kernel_spmd(nc, in_maps, core_ids=list(range(NC)))
    LAST_RUNS.append(res)
    return res


def _pack_wb(fo, Wb):
    # [k=1024, d] -> [p, jp, bank, plane, n]; k = (2*jp+plane)*128+p
    d = 32 * fo
    nb = d // 512
    _WB_CACHE[fo] = np.ascontiguousarray(
        Wb.reshape(4, 2, 128, nb, 512).transpose(2, 0, 3, 1, 4)
        .reshape(128, 8 * d)).astype(F8E4)


def kernel(**inputs):
    x = np.asarray(inputs["x"], np.float32)
    edge_index = np.asarray(inputs["edge_index"])
    eap = np.asarray(inputs["edge_attr_packed"])
    batch = np.asarray(inputs["batch"])
    W1a = np.asarray(inputs["W1a"], np.float32)
    W1b = np.asarray(inputs["W1b"], np.float32)
    W2a = np.asarray(inputs["W2a"], np.float32)
    W2b = np.asarray(inputs["W2b"], np.float32)
    b1a = np.asarray(inputs["b1a"], np.float32)
    b1b = np.asarray(inputs["b1b"], np.float32)
    b2a = np.asarray(inputs["b2a"], np.float32)
    b2b = np.asarray(inputs["b2b"], np.float32)
    root1 = np.asarray(inputs["root1"], np.float32)
    bias1 = np.asarray(inputs["bias1"], np.float32)
    root2 = np.asarray(inputs["root2"], np.float32)
    bias2 = np.asarray(inputs["bias2"], np.float32)

    LAST_RUNS.clear()
    sch = _Schedule(eap)
    _pack_wb(32, W1b)
    _pack_wb(64, W2b)

    # unique edge-attr bit patterns -> [U, 16] (MSB-first per byte)
    shifts = np.arange(15, -1, -1, dtype=np.int64)
    ea_u = ((sch.uniq_vals[:, None] >> shifts) & 1).astype(np.float32)

    src, dst = edge_index[0], edge_index[1]
    segmean_dst = _SegMean(dst, N)

    h1_u = _relu(ea_u @ W1a + b1a)
    h2_u = _relu(ea_u @ W2a + b2a)
    res = _run_both(sch, h1_u, h2_u)

    theta1 = _theta_slots(sch, res, "tho1", 32)
    msg1 = sch.contract(theta1, x[src], b1b, 32)
    h = _relu(segmean_dst(msg1) + x @ root1 + bias1)

    theta2 = _theta_slots(sch, res, "tho2", 64)
    msg2 = sch.contract(theta2, h[src], b2b, 64)
    h = _relu(segmean_dst(msg2) + h @ root2 + bias2)

    g = _SegMean(batch, NG)(h)
    g = _relu(g @ np.asarray(inputs["fcW1"], np.float32) + np.asarray(inputs["fcb1"], np.float32))
    g = _relu(g @ np.asarray(inputs["fcW2"], np.float32) + np.asarray(inputs["fcb2"], np.float32))
    g = _relu(g @ np.asarray(inputs["fcW3"], np.float32) + np.asarray(inputs["fcb3"], np.float32))
    return (g @ np.asarray(inputs["fcW4"], np.float32) + np.asarray(inputs["fcb4"], np.float32)).astype(np.float32)
